# revision 27
# baseline (speedup 1.0000x reference)
"""Supervised contrastive loss on 8 trn2 NeuronCores (Bass/Tile).

Full inputs -> full output. Sharding: rows of the (sorted-by-label,
per-core rolled) embedding matrix are split 1024/core. Each core
computes its 1024x8192 block of the similarity matrix against the full
embedding set in bf16 on the TensorEngine, reduces it to partial loss
terms; the host finishes the reduction in f64.

v3: the per-tile PSUM drain (the real bottleneck: every similarity
must be exp'd and row-summed by ACT or DVE at ~1 elem/lane/cycle) is
rebalanced:
  - ACT true-exp with free accum row-sums on cols [0:4096+XB)
  - DVE Schraudolph fast-exp (tensor_scalar f32->i16 bitcast bf16) on
    the rest, reduced by a seeded tensor_tensor_reduce (accumulator
    chaining kills the per-tile NAC reduce)
  - the window pass is collapsed: one STT computes zm = z*m AND its
    row-sum S; the Ln pass runs on zm directly (masked-out cols
    contribute ln(ns), corrected on the host with class counts), so
    its accum gives the masked A-sum for free
  - B = sum m*sim/T moved entirely to the host: sum_i e_i . c_{l_i}
    = sum_k ||class_sum_k||^2  (window spans the whole class)
  - PSUM as 2 x [128, 4096] halves (all 8 banks) to halve ACT/DVE
    instruction-fixed overheads; 4096-col exp instructions
  - input DMA split into 5 pieces across 5 engine queues, triggered
    immediately, so the first real matmul only waits on piece 0

Key algebra: with z_ij = exp(sim_ij) and ns_i = sum_{labels differ} z_ij,
  pair_loss_ij = ln(z_ij + ns_i) - sim_ij
Rows are sorted by label and rolled per-core so that all positives
(same-label columns) of each 128-row tile live in one 512-wide window
inside the first PSUM half. ln(z_ij*m_ij + ns_i) = ln(ns_i) for
masked-out columns, so the unmasked accumulated Ln sum A' satisfies
  A = A' - (WIN - cnt_i) * ln(ns_i).
"""

import math
import os
import sys

import numpy as np

for _p in ("/opt/trn_rl_repo", "/root/.axon_site/_ro/trn_rl_repo"):
    if os.path.isdir(_p) and _p not in sys.path:
        sys.path.append(_p)

B = 8192
D = 128
TEMP = 0.07
SCALE = 1.0 / TEMP
N_CORES = 8
R = B // N_CORES  # rows per core
P = 128  # partitions
CH = 2048  # PSUM chunk width (4 banks x 2 buffers = all 8)
XB = 1280  # ACT true-exp cols inside chunk 2; DVE fast-exps the rest
EXP_S0 = math.exp(SCALE)  # z_ii for a unit-norm row
# Schraudolph fast-exp constants: exp(SCALE*g) ~= bitcast_bf16(i16(g*A+Bc));
# Bc calibrated so the z-weighted mean ratio fast/true is 1.0 for
# unit-normalized gaussian embeddings (kills the systematic ns bias)
SCH_A = SCALE * 184.6649652337873
SCH_B = 16248.63


def _split_multi_waits(nc, mybir, max_waits=1):
    """Hoist excess per-instruction sync waits onto same-engine NoOps.

    This container's walrus rejects instructions carrying more than one
    sync wait ("Too many sync wait commands"); semantics are identical
    when the preceding NoOps on the same engine perform the waits.
    """
    n_new = 0
    for func in nc.m.functions:
        for block in func.blocks:
            il = block.instructions
            i = 0
            while i < len(il):
                inst = il[i]
                si = getattr(inst, "sync_info", None)
                ow = list(si.on_wait) if (si is not None and si.on_wait) else []
                if len(ow) > max_waits:
                    keep = ow[-max_waits:]
                    hoist = ow[:-max_waits]
                    nops = []
                    for w in hoist:
                        nop = mybir.InstNoOp(
                            name=f"{inst.name}-ws{len(nops)}",
                            engine=inst.engine,
                            ins=[],
                            outs=[],
                            sync_info=mybir.SyncInfo(on_wait=[w], on_update=[]),
                        )
                        nops.append(nop)
                        n_new += 1
                    inst.sync_info = mybir.SyncInfo(
                        on_wait=keep,
                        on_update=list(si.on_update) if si.on_update else [],
                    )
                    il[i:i] = nops
                    i += len(nops)
                i += 1
    return n_new


def _build_program(WIN: int, OFF: int):
    import concourse.bass as bass
    import concourse.tile as tile
    from concourse import mybir

    f32 = mybir.dt.float32
    bf16 = mybir.dt.bfloat16
    i16 = mybir.dt.int16
    AF = mybir.ActivationFunctionType
    OP = mybir.AluOpType

    nc = bass.Bass()
    NRT = R // P  # 8 row tiles owned by this core
    NCH = B // CH  # 4 chunks per tile
    half = (WIN - P) // 2  # window margin each side of the 128 rows
    assert OFF - half >= 0
    assert OFF + (NRT - 1) * P - half + WIN <= CH, "window exceeds chunk 0"
    assert OFF + NRT * P <= 2048, "lhsT columns exceed first two pieces"

    # input pieces arrive as separate contiguous dram tensors so each DMA
    # runs with maximal descriptors (strided slices of one big tensor cost
    # a descriptor per 2-4KB row run)
    PIECES = [(0, 512), (512, 1024), (1024, 2048), (2048, 4096), (4096, 6144), (6144, 8192)]
    d_embp = [
        nc.dram_tensor(f"emb{i}", [P, b - a], bf16, kind="ExternalInput")
        for i, (a, b) in enumerate(PIECES)
    ]
    # per-row-tile mask slices, one tensor each, so tile rt's window op
    # only waits on its own 128KB
    d_mskt = [
        nc.dram_tensor(f"msk{rt}", [P, WIN], bf16, kind="ExternalInput")
        for rt in range(NRT)
    ]
    # per-(partition, row-tile) A' | ns partials; host does the rest
    d_out = nc.dram_tensor("out", [P, 2 * (R // P)], f32, kind="ExternalOutput")

    with tile.TileContext(nc) as tc:
        with (
            tc.tile_pool(name="emb", bufs=1) as pE,
            tc.tile_pool(name="consts", bufs=1) as pC,
            tc.tile_pool(name="parts", bufs=1) as pP,
            tc.tile_pool(name="d0", bufs=2) as pD0,
            tc.tile_pool(name="dd", bufs=2) as pD,
            tc.tile_pool(name="di", bufs=2) as pDI,
            tc.tile_pool(name="db", bufs=2) as pDB,
            tc.tile_pool(name="fw", bufs=2) as pF,
            tc.tile_pool(name="ps", bufs=2, space="PSUM") as psP,
        ):
            # ---------------- load ----------------
            # warm-up memset first so the PE warm-up chain never waits on
            # a DMA trigger; then spread the 5 emb pieces across 5 engine
            # queues (serialized triggers cost ~650ns each).
            wm = pC.tile([P, 512], bf16, tag="wm")
            nc.gpsimd.memset(wm, 0.0)

            eP = []
            trig = [nc.gpsimd, nc.scalar, nc.sync, nc.gpsimd, nc.scalar, nc.sync]
            for idx, (a, b) in enumerate(PIECES):
                t = pE.tile([P, b - a], bf16, tag=f"e{idx}")
                trig[idx].dma_start(out=t, in_=d_embp[idx][:, :])
                eP.append(t)
            mskT = []
            mtrig = [nc.gpsimd, nc.scalar, nc.sync]
            for rt in range(NRT):
                t = pC.tile([P, WIN], bf16, tag=f"msk{rt}")
                mtrig[rt % 3].dma_start(out=t, in_=d_mskt[rt][:, :])
                mskT.append(t)

            def ecols(c0, c1):
                """AP over eT cols [c0:c1) — must lie inside one piece."""
                for idx, (a, b) in enumerate(PIECES):
                    if a <= c0 and c1 <= b:
                        return eP[idx][:, c0 - a : c1 - a]
                raise AssertionError(f"cols {c0}:{c1} cross pieces")

            # ---------------- PE warm-up ----------------
            # HAM clock-gates the PE to 1.2GHz until it sees ~3.4us of
            # sustained busy. Burn dummy matmuls on a scratch tile during
            # the DMA wait so the main loop enters at 2.4GHz.
            gw = psP.tile([P, CH], f32, tag="g")
            for _ in range(8):
                nc.tensor.matmul(
                    gw[:, :512],
                    lhsT=wm[:, :P],
                    rhs=wm,
                    start=True,
                    stop=True,
                )

            # per-row-tile accumulators: 4 adjacent slots per tile (ACT
            # chunk sums x3, fast-exp-minus-S) so one tensor_reduce
            # produces ns directly
            NAC = 4
            S8_t = pP.tile([P, NRT], f32, tag="S8")
            tot8 = pP.tile([P, NRT], f32, tag="tot8")
            acc = pP.tile([P, NRT * NAC], f32, tag="acc")
            out16 = pP.tile([P, 2 * NRT], f32, tag="out16")
            A8 = out16[:, 0:NRT]  # unmasked ln sums  (A')
            ns8 = out16[:, NRT : 2 * NRT]

            # ---------------- main loop over this core's row tiles ----------------
            prev = None  # (rt, zm' handle) pending ln pass
            for rt in range(NRT):
                row0 = OFF + rt * P
                c0 = row0 - half  # window start column (inside chunk 0)
                lhsT_e = ecols(row0, row0 + P)
                m_rt = mskT[rt]

                a0 = rt * NAC  # this tile's accumulator slots

                for ci in range(NCH):
                    g = psP.tile([P, CH], f32, tag="g")
                    for s in range(0, CH, 512):
                        a = ci * CH + s
                        nc.tensor.matmul(
                            g[:, s : s + 512],
                            lhsT=lhsT_e,
                            rhs=ecols(a, a + 512),
                            start=True,
                            stop=True,
                        )
                    if ci == 0:
                        # true exp (window lives here) with free row-sum
                        d0 = pD0.tile([P, CH], bf16, tag="d0")
                        nc.scalar.activation(
                            d0, g, AF.Exp, scale=SCALE,
                            accum_out=acc[:, a0 : a0 + 1],
                        )
                        # window pass: zm = z*m with S accum (one STT)
                        zm = pDB.tile([P, WIN], bf16, tag="zm")
                        nc.vector.scalar_tensor_tensor(
                            out=zm,
                            in0=d0[:, c0 : c0 + WIN],
                            scalar=1.0,
                            in1=m_rt,
                            op0=OP.mult,
                            op1=OP.mult,
                            accum_out=S8_t[:, rt : rt + 1],
                        )
                    elif ci == 1:
                        dd = pD.tile([P, CH], bf16, tag="dd")
                        nc.scalar.activation(
                            dd, g, AF.Exp, scale=SCALE,
                            accum_out=acc[:, a0 + 1 : a0 + 2],
                        )
                    elif ci == 2:
                        # split: ACT true-exp on [0:XB), DVE fast-exp rest;
                        # both fast-exp chunks land in one tile so a single
                        # reduce sums them (fewer DVE ops and semaphores)
                        dIJ = pDI.tile([P, 2 * CH - XB], i16, tag="dij")
                        nc.vector.tensor_scalar(
                            out=dIJ[:, 0 : CH - XB],
                            in0=g[:, XB:CH],
                            scalar1=SCH_A,
                            scalar2=SCH_B,
                            op0=OP.mult,
                            op1=OP.add,
                        )
                        dd = pD.tile([P, XB], bf16, tag="dd2")
                        nc.scalar.activation(
                            dd, g[:, 0:XB], AF.Exp, scale=SCALE,
                            accum_out=acc[:, a0 + 2 : a0 + 3],
                        )
                    else:
                        nc.vector.tensor_scalar(
                            out=dIJ[:, CH - XB : 2 * CH - XB],
                            in0=g,
                            scalar1=SCH_A,
                            scalar2=SCH_B,
                            op0=OP.mult,
                            op1=OP.add,
                        )
                        nc.vector.tensor_reduce(
                            acc[:, a0 + 3 : a0 + 4],
                            dIJ[:, :].bitcast(bf16),
                            axis=mybir.AxisListType.X,
                            op=OP.add,
                        )

                # ns assembly: tot = sum of the 5 slots; ns = tot - S
                # (subtract on the idle Pool engine)
                nc.vector.tensor_reduce(
                    tot8[:, rt : rt + 1],
                    acc[:, a0 : a0 + NAC],
                    axis=mybir.AxisListType.X,
                    op=OP.add,
                )
                nc.gpsimd.tensor_tensor(
                    ns8[:, rt : rt + 1],
                    tot8[:, rt : rt + 1],
                    S8_t[:, rt : rt + 1],
                    op=OP.subtract,
                )

                # ln pass for the PREVIOUS tile: its ns is ready by now, so
                # the ACT engine never stalls on the DVE work.
                if prev is not None:
                    prt, pzm = prev
                    fw = pF.tile([P, WIN], bf16, tag="fw")
                    nc.scalar.activation(
                        fw,
                        pzm,
                        AF.Ln,
                        bias=ns8[:, prt : prt + 1],
                        scale=1.0,
                        accum_out=A8[:, prt : prt + 1],
                    )
                prev = (rt, zm)

            # ns partials are final once the loop ends; ship them while
            # the last ln pass still runs
            nc.sync.dma_start(out=d_out[:, NRT : 2 * NRT], in_=ns8)

            # final pending ln pass
            prt, pzm = prev
            fw = pF.tile([P, WIN], bf16, tag="fw")
            nc.scalar.activation(
                fw,
                pzm,
                AF.Ln,
                bias=ns8[:, prt : prt + 1],
                scale=1.0,
                accum_out=A8[:, prt : prt + 1],
            )

            # ---------------- tail ----------------
            nc.sync.dma_start(out=d_out[:, 0:NRT], in_=A8)

    _split_multi_waits(nc, mybir)
    return nc


def _plan(labels: np.ndarray):
    """Sort-by-label order, window geometry."""
    order = np.argsort(labels, kind="stable")
    counts = np.bincount(labels)
    max_cls = int(counts.max()) if counts.size else 1
    # per-row-tile window: 128 rows + margin >= max_cls-1 each side
    win = 512
    while win < B and (win - P) // 2 < max_cls - 1:
        win += 512
    win = min(win, 1024)  # window must fit inside the ACT half
    off = max(256, (win - P) // 2 + 64)
    assert (win - P) // 2 >= max_cls - 1, "class too large"
    return order, counts, off, win


def _host_inputs(emb, lab, order, off, win):
    import ml_dtypes

    half = (win - P) // 2
    # L2-normalize rows on host (matches F.normalize with eps=1e-12)
    norm = np.linalg.norm(emb, axis=1, keepdims=True)
    e = emb / np.maximum(norm, 1e-12)
    pieces = [(0, 512), (512, 1024), (1024, 2048), (2048, 4096), (4096, 6144), (6144, 8192)]
    in_maps = []
    for k in range(N_CORES):
        ck = np.roll(order, off - R * k)
        lab_r = lab[ck]
        # pre-transposed [D, B] so no on-device transposes are needed;
        # each column piece ships as its own contiguous tensor
        eT = e[ck].T.astype(ml_dtypes.bfloat16)
        im = {
            f"emb{i}": np.ascontiguousarray(eT[:, a:b])
            for i, (a, b) in enumerate(pieces)
        }
        # per-row-tile same-label masks over each tile's window
        for rt in range(R // P):
            row0 = off + rt * P
            c0 = row0 - half
            rl = lab_r[row0 : row0 + P]
            cl = lab_r[c0 : c0 + win]
            m = (rl[:, None] == cl[None, :]).astype(np.float32)
            im[f"msk{rt}"] = np.ascontiguousarray(m.astype(ml_dtypes.bfloat16))
        in_maps.append(im)
    return in_maps


def kernel(embeddings: np.ndarray, labels: np.ndarray) -> np.ndarray:
    from concourse.bass_utils import run_bass_kernel_spmd

    emb = np.ascontiguousarray(np.asarray(embeddings, dtype=np.float32))
    lab = np.asarray(labels).astype(np.int64).ravel()
    assert emb.shape == (B, D) and lab.shape == (B,)

    order, counts, off, win = _plan(lab)
    in_maps = _host_inputs(emb, lab, order, off, win)

    nc = _build_program(win, off)
    res = run_bass_kernel_spmd(nc, in_maps, core_ids=list(range(N_CORES)))

    # host reduction in f64:
    #   loss_sum = sum_i [ 1/T + A_i - fd_i ] - SCALE * sum_k ||c_k||^2
    #   A_i = A'_i - (WIN - cnt_i) * ln(ns_i),  fd_i = ln(ns_i + e^{1/T})
    norm = np.linalg.norm(emb, axis=1, keepdims=True)
    e64 = (emb / np.maximum(norm, 1e-12)).astype(np.float64)
    csum = np.zeros((int(lab.max()) + 1, D), dtype=np.float64)
    np.add.at(csum, lab, e64)
    B_host = SCALE * float((csum * csum).sum())

    NRT = R // P
    loss_sum = SCALE * B - B_host
    for k, r in enumerate(res.results):
        o = np.asarray(r["out"], dtype=np.float64)
        a8, ns8 = o[:, :NRT], o[:, NRT:]
        ck = np.roll(order, off - R * k)
        lab_r = lab[ck]
        # cnt[p, rt] = class size of the row at (partition p, tile rt)
        rows = off + np.arange(NRT)[None, :] * P + np.arange(P)[:, None]
        cnt = counts[lab_r[rows]]
        lns = np.log(ns8)
        a = a8 - (win - cnt) * lns
        fd = np.log(ns8 + EXP_S0)
        loss_sum += float((a - fd).sum())

    n_c = counts[lab]
    valid = (n_c >= 2) & (n_c <= B - 1)
    valid_count = int((n_c - 1)[valid].sum())
    loss = loss_sum / valid_count if valid_count > 0 else 0.0
    return np.asarray([loss], dtype=np.float32)


# revision 28
# speedup vs baseline: 1.1976x; 1.1976x over previous
"""Supervised contrastive loss on 8 trn2 NeuronCores (Bass/Tile).

Full inputs -> full output. Sharding: rows of the (sorted-by-label,
per-core rolled) embedding matrix are split 1024/core. Each core
computes its 1024x8192 block of the similarity matrix against the full
embedding set in bf16 on the TensorEngine, reduces it to partial loss
terms; the host finishes the reduction in f64.

v3: the per-tile PSUM drain (the real bottleneck: every similarity
must be exp'd and row-summed by ACT or DVE at ~1 elem/lane/cycle) is
rebalanced:
  - ACT true-exp with free accum row-sums on cols [0:4096+XB)
  - DVE Schraudolph fast-exp (tensor_scalar f32->i16 bitcast bf16) on
    the rest, reduced by a seeded tensor_tensor_reduce (accumulator
    chaining kills the per-tile NAC reduce)
  - the window pass is collapsed: one STT computes zm = z*m AND its
    row-sum S; the Ln pass runs on zm directly (masked-out cols
    contribute ln(ns), corrected on the host with class counts), so
    its accum gives the masked A-sum for free
  - B = sum m*sim/T moved entirely to the host: sum_i e_i . c_{l_i}
    = sum_k ||class_sum_k||^2  (window spans the whole class)
  - PSUM as 2 x [128, 4096] halves (all 8 banks) to halve ACT/DVE
    instruction-fixed overheads; 4096-col exp instructions
  - input DMA split into 5 pieces across 5 engine queues, triggered
    immediately, so the first real matmul only waits on piece 0

Key algebra: with z_ij = exp(sim_ij) and ns_i = sum_{labels differ} z_ij,
  pair_loss_ij = ln(z_ij + ns_i) - sim_ij
Rows are sorted by label and rolled per-core so that all positives
(same-label columns) of each 128-row tile live in one 512-wide window
inside the first PSUM half. ln(z_ij*m_ij + ns_i) = ln(ns_i) for
masked-out columns, so the unmasked accumulated Ln sum A' satisfies
  A = A' - (WIN - cnt_i) * ln(ns_i).
"""

import math
import os
import sys

import numpy as np

for _p in ("/opt/trn_rl_repo", "/root/.axon_site/_ro/trn_rl_repo"):
    if os.path.isdir(_p) and _p not in sys.path:
        sys.path.append(_p)

B = 8192
D = 128
TEMP = 0.07
SCALE = 1.0 / TEMP
N_CORES = 8
R = B // N_CORES  # rows per core
P = 128  # partitions
CH = 2048  # PSUM chunk width (4 banks x 2 buffers = all 8)
XB = 1280  # ACT true-exp cols inside chunk 2; DVE fast-exps the rest
EXP_S0 = math.exp(SCALE)  # z_ii for a unit-norm row
# Schraudolph fast-exp constants: exp(SCALE*g) ~= bitcast_bf16(i16(g*A+Bc));
# Bc calibrated so the z-weighted mean ratio fast/true is 1.0 for
# unit-normalized gaussian embeddings (kills the systematic ns bias)
SCH_A = SCALE * 184.6649652337873
SCH_B = 16248.63


def _split_multi_waits(nc, mybir, max_waits=1):
    """Hoist excess per-instruction sync waits onto same-engine NoOps.

    This container's walrus rejects instructions carrying more than one
    sync wait ("Too many sync wait commands"); semantics are identical
    when the preceding NoOps on the same engine perform the waits.
    """
    n_new = 0
    for func in nc.m.functions:
        for block in func.blocks:
            il = block.instructions
            i = 0
            while i < len(il):
                inst = il[i]
                si = getattr(inst, "sync_info", None)
                ow = list(si.on_wait) if (si is not None and si.on_wait) else []
                if len(ow) > max_waits:
                    keep = ow[-max_waits:]
                    hoist = ow[:-max_waits]
                    nops = []
                    for w in hoist:
                        nop = mybir.InstNoOp(
                            name=f"{inst.name}-ws{len(nops)}",
                            engine=inst.engine,
                            ins=[],
                            outs=[],
                            sync_info=mybir.SyncInfo(on_wait=[w], on_update=[]),
                        )
                        nops.append(nop)
                        n_new += 1
                    inst.sync_info = mybir.SyncInfo(
                        on_wait=keep,
                        on_update=list(si.on_update) if si.on_update else [],
                    )
                    il[i:i] = nops
                    i += len(nops)
                i += 1
    return n_new


def _build_program(WIN: int, OFF: int):
    import concourse.bass as bass
    import concourse.tile as tile
    from concourse import mybir

    f32 = mybir.dt.float32
    bf16 = mybir.dt.bfloat16
    i16 = mybir.dt.int16
    AF = mybir.ActivationFunctionType
    OP = mybir.AluOpType

    nc = bass.Bass()
    NRT = R // P  # 8 row tiles owned by this core
    NCH = B // CH  # 4 chunks per tile
    half = (WIN - P) // 2  # window margin each side of the 128 rows
    assert OFF - half >= 0
    assert OFF + (NRT - 1) * P - half + WIN <= CH, "window exceeds chunk 0"
    assert OFF + NRT * P <= 2048, "lhsT columns exceed first two pieces"

    # input pieces arrive as separate contiguous dram tensors so each DMA
    # runs with maximal descriptors (strided slices of one big tensor cost
    # a descriptor per 2-4KB row run)
    PIECES = [(0, 512), (512, 1024), (1024, 2048), (2048, 4096), (4096, 6144), (6144, 8192)]
    d_embp = [
        nc.dram_tensor(f"emb{i}", [P, b - a], bf16, kind="ExternalInput")
        for i, (a, b) in enumerate(PIECES)
    ]
    # per-row-tile mask slices, one tensor each, so tile rt's window op
    # only waits on its own 128KB
    d_mskt = [
        nc.dram_tensor(f"msk{rt}", [P, WIN], bf16, kind="ExternalInput")
        for rt in range(NRT)
    ]
    # per-(partition, row-tile) A' | ns partials; host does the rest
    d_out = nc.dram_tensor("out", [P, 2 * (R // P)], f32, kind="ExternalOutput")

    with tile.TileContext(nc) as tc:
        with (
            tc.tile_pool(name="emb", bufs=1) as pE,
            tc.tile_pool(name="consts", bufs=1) as pC,
            tc.tile_pool(name="parts", bufs=1) as pP,
            tc.tile_pool(name="d0", bufs=2) as pD0,
            tc.tile_pool(name="dd", bufs=2) as pD,
            tc.tile_pool(name="di", bufs=2) as pDI,
            tc.tile_pool(name="db", bufs=2) as pDB,
            tc.tile_pool(name="fw", bufs=2) as pF,
            tc.tile_pool(name="ps", bufs=2, space="PSUM") as psP,
        ):
            # ---------------- load ----------------
            # warm-up memset first so the PE warm-up chain never waits on
            # a DMA trigger; then spread the 5 emb pieces across 5 engine
            # queues (serialized triggers cost ~650ns each).
            wm = pC.tile([P, 512], bf16, tag="wm")
            nc.gpsimd.memset(wm, 0.0)

            eP = []
            trig = [nc.gpsimd, nc.scalar, nc.sync, nc.gpsimd, nc.scalar, nc.sync]
            for idx, (a, b) in enumerate(PIECES):
                t = pE.tile([P, b - a], bf16, tag=f"e{idx}")
                trig[idx].dma_start(out=t, in_=d_embp[idx][:, :])
                eP.append(t)
            mskT = []
            mtrig = [nc.gpsimd, nc.scalar, nc.sync]
            for rt in range(NRT):
                t = pC.tile([P, WIN], bf16, tag=f"msk{rt}")
                mtrig[rt % 3].dma_start(out=t, in_=d_mskt[rt][:, :])
                mskT.append(t)

            def ecols(c0, c1):
                """AP over eT cols [c0:c1) — must lie inside one piece."""
                for idx, (a, b) in enumerate(PIECES):
                    if a <= c0 and c1 <= b:
                        return eP[idx][:, c0 - a : c1 - a]
                raise AssertionError(f"cols {c0}:{c1} cross pieces")

            # ---------------- PE warm-up ----------------
            # HAM clock-gates the PE to 1.2GHz until it sees ~3.4us of
            # sustained busy. Burn dummy matmuls on a scratch tile during
            # the DMA wait so the main loop enters at 2.4GHz.
            gw = psP.tile([P, CH], f32, tag="g")
            for _ in range(8):
                nc.tensor.matmul(
                    gw[:, :512],
                    lhsT=wm[:, :P],
                    rhs=wm,
                    start=True,
                    stop=True,
                )

            # per-row-tile accumulators: 4 adjacent slots per tile (ACT
            # chunk sums x3, fast-exp-minus-S) so one tensor_reduce
            # produces ns directly
            NAC = 4
            S8_t = pP.tile([P, NRT], f32, tag="S8")
            tot8 = pP.tile([P, NRT], f32, tag="tot8")
            acc = pP.tile([P, NRT * NAC], f32, tag="acc")
            out16 = pP.tile([P, 2 * NRT], f32, tag="out16")
            A8 = out16[:, 0:NRT]  # unmasked ln sums  (A')
            ns8 = out16[:, NRT : 2 * NRT]

            # ---------------- main loop over this core's row tiles ----------------
            prev = None  # (rt, zm' handle) pending ln pass
            for rt in range(NRT):
                row0 = OFF + rt * P
                c0 = row0 - half  # window start column (inside chunk 0)
                lhsT_e = ecols(row0, row0 + P)
                m_rt = mskT[rt]

                a0 = rt * NAC  # this tile's accumulator slots

                for ci in range(NCH):
                    g = psP.tile([P, CH], f32, tag="g")
                    for s in range(0, CH, 512):
                        a = ci * CH + s
                        nc.tensor.matmul(
                            g[:, s : s + 512],
                            lhsT=lhsT_e,
                            rhs=ecols(a, a + 512),
                            start=True,
                            stop=True,
                        )
                    if ci == 0:
                        # true exp (window lives here) with free row-sum
                        d0 = pD0.tile([P, CH], bf16, tag="d0")
                        nc.scalar.activation(
                            d0, g, AF.Exp, scale=SCALE,
                            accum_out=acc[:, a0 : a0 + 1],
                        )
                        # window pass: zm = z*m with S accum (one STT)
                        zm = pDB.tile([P, WIN], bf16, tag="zm")
                        nc.vector.scalar_tensor_tensor(
                            out=zm,
                            in0=d0[:, c0 : c0 + WIN],
                            scalar=1.0,
                            in1=m_rt,
                            op0=OP.mult,
                            op1=OP.mult,
                            accum_out=S8_t[:, rt : rt + 1],
                        )
                    elif ci == 1:
                        dd = pD.tile([P, CH], bf16, tag="dd")
                        nc.scalar.activation(
                            dd, g, AF.Exp, scale=SCALE,
                            accum_out=acc[:, a0 + 1 : a0 + 2],
                        )
                    elif ci == 2:
                        # split: ACT true-exp on [0:XB), DVE fast-exp rest;
                        # both fast-exp chunks land in one tile so a single
                        # reduce sums them (fewer DVE ops and semaphores)
                        dIJ = pDI.tile([P, 2 * CH - XB], i16, tag="dij")
                        nc.vector.tensor_scalar(
                            out=dIJ[:, 0 : CH - XB],
                            in0=g[:, XB:CH],
                            scalar1=SCH_A,
                            scalar2=SCH_B,
                            op0=OP.mult,
                            op1=OP.add,
                        )
                        dd = pD.tile([P, XB], bf16, tag="dd2")
                        nc.scalar.activation(
                            dd, g[:, 0:XB], AF.Exp, scale=SCALE,
                            accum_out=acc[:, a0 + 2 : a0 + 3],
                        )
                    else:
                        nc.vector.tensor_scalar(
                            out=dIJ[:, CH - XB : 2 * CH - XB],
                            in0=g,
                            scalar1=SCH_A,
                            scalar2=SCH_B,
                            op0=OP.mult,
                            op1=OP.add,
                        )
                        nc.vector.tensor_reduce(
                            acc[:, a0 + 3 : a0 + 4],
                            dIJ[:, :].bitcast(bf16),
                            axis=mybir.AxisListType.X,
                            op=OP.add,
                        )

                # ns assembly: tot = sum of the 5 slots; ns = tot - S
                # (subtract on the idle Pool engine)
                nc.vector.tensor_reduce(
                    tot8[:, rt : rt + 1],
                    acc[:, a0 : a0 + NAC],
                    axis=mybir.AxisListType.X,
                    op=OP.add,
                )
                nc.gpsimd.tensor_tensor(
                    ns8[:, rt : rt + 1],
                    tot8[:, rt : rt + 1],
                    S8_t[:, rt : rt + 1],
                    op=OP.subtract,
                )

                # ln pass for the PREVIOUS tile: its ns is ready by now, so
                # the ACT engine never stalls on the DVE work.
                if prev is not None:
                    prt, pzm = prev
                    fw = pF.tile([P, WIN], bf16, tag="fw")
                    nc.scalar.activation(
                        fw,
                        pzm,
                        AF.Ln,
                        bias=ns8[:, prt : prt + 1],
                        scale=1.0,
                        accum_out=A8[:, prt : prt + 1],
                    )
                prev = (rt, zm)

            # ns partials and A' for tiles 0..NRT-2 are final once the
            # loop ends; ship them while the last ln pass still runs, so
            # only a 512B transfer remains on the critical path
            nc.sync.dma_start(out=d_out[:, NRT : 2 * NRT], in_=ns8)
            nc.scalar.dma_start(
                out=d_out[:, 0 : NRT - 1], in_=A8[:, 0 : NRT - 1]
            )

            # final pending ln pass
            prt, pzm = prev
            fw = pF.tile([P, WIN], bf16, tag="fw")
            nc.scalar.activation(
                fw,
                pzm,
                AF.Ln,
                bias=ns8[:, prt : prt + 1],
                scale=1.0,
                accum_out=A8[:, prt : prt + 1],
            )

            # ---------------- tail ----------------
            nc.sync.dma_start(
                out=d_out[:, NRT - 1 : NRT], in_=A8[:, NRT - 1 : NRT]
            )

    _split_multi_waits(nc, mybir)
    return nc


def _plan(labels: np.ndarray):
    """Sort-by-label order, window geometry."""
    order = np.argsort(labels, kind="stable")
    counts = np.bincount(labels)
    max_cls = int(counts.max()) if counts.size else 1
    # per-row-tile window: 128 rows + margin >= max_cls-1 each side
    win = 512
    while win < B and (win - P) // 2 < max_cls - 1:
        win += 512
    win = min(win, 1024)  # window must fit inside the ACT half
    off = max(256, (win - P) // 2 + 64)
    assert (win - P) // 2 >= max_cls - 1, "class too large"
    return order, counts, off, win


def _host_inputs(emb, lab, order, off, win):
    import ml_dtypes

    half = (win - P) // 2
    # L2-normalize rows on host (matches F.normalize with eps=1e-12)
    norm = np.linalg.norm(emb, axis=1, keepdims=True)
    e = emb / np.maximum(norm, 1e-12)
    pieces = [(0, 512), (512, 1024), (1024, 2048), (2048, 4096), (4096, 6144), (6144, 8192)]
    in_maps = []
    for k in range(N_CORES):
        ck = np.roll(order, off - R * k)
        lab_r = lab[ck]
        # pre-transposed [D, B] so no on-device transposes are needed;
        # each column piece ships as its own contiguous tensor
        eT = e[ck].T.astype(ml_dtypes.bfloat16)
        im = {
            f"emb{i}": np.ascontiguousarray(eT[:, a:b])
            for i, (a, b) in enumerate(pieces)
        }
        # per-row-tile same-label masks over each tile's window
        for rt in range(R // P):
            row0 = off + rt * P
            c0 = row0 - half
            rl = lab_r[row0 : row0 + P]
            cl = lab_r[c0 : c0 + win]
            m = (rl[:, None] == cl[None, :]).astype(np.float32)
            im[f"msk{rt}"] = np.ascontiguousarray(m.astype(ml_dtypes.bfloat16))
        in_maps.append(im)
    return in_maps


def kernel(embeddings: np.ndarray, labels: np.ndarray) -> np.ndarray:
    from concourse.bass_utils import run_bass_kernel_spmd

    emb = np.ascontiguousarray(np.asarray(embeddings, dtype=np.float32))
    lab = np.asarray(labels).astype(np.int64).ravel()
    assert emb.shape == (B, D) and lab.shape == (B,)

    order, counts, off, win = _plan(lab)
    in_maps = _host_inputs(emb, lab, order, off, win)

    nc = _build_program(win, off)
    res = run_bass_kernel_spmd(nc, in_maps, core_ids=list(range(N_CORES)))

    # host reduction in f64:
    #   loss_sum = sum_i [ 1/T + A_i - fd_i ] - SCALE * sum_k ||c_k||^2
    #   A_i = A'_i - (WIN - cnt_i) * ln(ns_i),  fd_i = ln(ns_i + e^{1/T})
    norm = np.linalg.norm(emb, axis=1, keepdims=True)
    e64 = (emb / np.maximum(norm, 1e-12)).astype(np.float64)
    csum = np.zeros((int(lab.max()) + 1, D), dtype=np.float64)
    np.add.at(csum, lab, e64)
    B_host = SCALE * float((csum * csum).sum())

    NRT = R // P
    loss_sum = SCALE * B - B_host
    for k, r in enumerate(res.results):
        o = np.asarray(r["out"], dtype=np.float64)
        a8, ns8 = o[:, :NRT], o[:, NRT:]
        ck = np.roll(order, off - R * k)
        lab_r = lab[ck]
        # cnt[p, rt] = class size of the row at (partition p, tile rt)
        rows = off + np.arange(NRT)[None, :] * P + np.arange(P)[:, None]
        cnt = counts[lab_r[rows]]
        lns = np.log(ns8)
        a = a8 - (win - cnt) * lns
        fd = np.log(ns8 + EXP_S0)
        loss_sum += float((a - fd).sum())

    n_c = counts[lab]
    valid = (n_c >= 2) & (n_c <= B - 1)
    valid_count = int((n_c - 1)[valid].sum())
    loss = loss_sum / valid_count if valid_count > 0 else 0.0
    return np.asarray([loss], dtype=np.float32)
